# revision 1
# baseline (speedup 1.0000x reference)
"""GCN (3-layer, PyG GCNConv-style) forward on 8 Trainium2 NeuronCores.

Strategy: data-parallel over the 64 graphs (8 graphs per core).  The
message-passing scatter-add is expressed as a dense normalized-adjacency
matmul: for each graph we build A^T (2048x2048, fp32 rounded to fp32r
encoding) on the host from edge_index, ship it to device HBM, and keep it
SBUF-resident across all GCN layers.  Per layer on device:
    h   = x @ W          (16 matmuls, node-major output)
    x'  = relu(A @ h + b) (64 fp32r matmuls, N=512, feature-major output)
The feature gather from the 500k-row embedding table runs on device via
indirect DMA (128 rows per descriptor batch).  The layer orientations
alternate (node-major <-> feature-major) so no per-layer transposes are
needed; only the initial gathered features are transposed once via the PE.
"""

import os
import sys
import time

for _p in ("/opt/trn_rl_repo", "/root/.axon_site/_ro/trn_rl_repo"):
    if os.path.isdir(_p) and _p not in sys.path:
        sys.path.insert(0, _p)

import numpy as np

import concourse.bass as bass
import concourse.bacc as bacc
import concourse.mybir as mybir
import concourse.tile as tile
from concourse import bass2jax

G, N, E = 64, 2048, 32768
D = H = 128
O = 2
ALL = 500_000
P = 128
N_CORES = 8
GPC = G // N_CORES          # graphs per core
NCH = N // P                # 128-row chunks per graph (16)

f32 = mybir.dt.float32
f32r = mybir.dt.float32r
i32 = mybir.dt.int32


def _round_f32r(a: np.ndarray) -> np.ndarray:
    """Round fp32 to the fp32r value set (11-bit mantissa, round-to-nearest).

    Matches the rounding the DVE applies when producing fp32r (verified on
    hardware up to ties).  Mantissa carry propagates into the exponent via
    integer add.
    """
    u = np.ascontiguousarray(a, np.float32).view(np.uint32).copy()
    u += 0x800          # finite inputs only: no wrap past the exponent field
    u &= 0xFFFF_F000
    return u.view(np.float32)


def _build_program(n_layers: int):
    nc = bacc.Bacc("TRN2", target_bir_lowering=False, debug=False,
                   num_devices=N_CORES)

    table = nc.dram_tensor("table", [ALL, D], f32, kind="ExternalInput")
    fidx = nc.dram_tensor("fidx", [P, GPC * NCH], i32, kind="ExternalInput")
    at = nc.dram_tensor("at", [GPC * N, N], f32r, kind="ExternalInput")
    wres = nc.dram_tensor("wres", [D, H], f32r, kind="ExternalInput")
    bres = nc.dram_tensor("bres", [H, 1], f32, kind="ExternalInput")
    gw = nc.dram_tensor("gw", [n_layers, H, H], f32r, kind="ExternalInput")
    gb = nc.dram_tensor("gb", [H, n_layers], f32, kind="ExternalInput")
    wfc = nc.dram_tensor("wfc", [H, H], f32r, kind="ExternalInput")
    bfc = nc.dram_tensor("bfc", [H, 1], f32, kind="ExternalInput")
    wlin = nc.dram_tensor("wlin", [H, O], f32, kind="ExternalInput")
    lbb = nc.dram_tensor("lbb", [GPC, O], f32, kind="ExternalInput")
    out_ls = nc.dram_tensor("out_ls", [GPC, O], f32, kind="ExternalOutput")
    out_lg = nc.dram_tensor("out_lg", [GPC, O], f32, kind="ExternalOutput")

    from concourse.masks import make_identity

    with tile.TileContext(nc) as tc:
        with tc.tile_pool(name="const", bufs=1) as const, \
             tc.tile_pool(name="apool", bufs=1) as apool, \
             tc.tile_pool(name="xpool", bufs=2) as xpool, \
             tc.tile_pool(name="hpool", bufs=1) as hpool, \
             tc.tile_pool(name="fpool", bufs=3) as fpool, \
             tc.tile_pool(name="wps", bufs=2, space="PSUM") as wps, \
             tc.tile_pool(name="tps", bufs=2, space="PSUM") as tps, \
             tc.tile_pool(name="aps", bufs=1, space="PSUM") as aps:

            # ---- constants ----
            fidx_sb = const.tile([P, GPC * NCH], i32)
            nc.sync.dma_start(out=fidx_sb[:], in_=fidx[:])
            wres_sb = const.tile([D, H], f32r)
            nc.sync.dma_start(out=wres_sb[:], in_=wres[:])
            gw_sb = const.tile([H, n_layers * H], f32r)
            for l in range(n_layers):
                nc.sync.dma_start(out=gw_sb[:, l * H:(l + 1) * H], in_=gw[l])
            gb_sb = const.tile([H, n_layers], f32)
            nc.sync.dma_start(out=gb_sb[:], in_=gb[:])
            bres_sb = const.tile([H, 1], f32)
            nc.sync.dma_start(out=bres_sb[:], in_=bres[:])
            wfc_sb = const.tile([H, H], f32r)
            nc.sync.dma_start(out=wfc_sb[:], in_=wfc[:])
            bfc_sb = const.tile([H, 1], f32)
            nc.sync.dma_start(out=bfc_sb[:], in_=bfc[:])
            wlin_sb = const.tile([H, O], f32)
            nc.sync.dma_start(out=wlin_sb[:], in_=wlin[:])
            lbb_sb = const.tile([GPC, O], f32)
            nc.sync.dma_start(out=lbb_sb[:], in_=lbb[:])
            ident = const.tile([P, P], f32)
            make_identity(nc, ident[:])
            macc = const.tile([P, GPC * 4], f32)
            means = const.tile([P, GPC], f32)

            for g in range(GPC):
                # ---- stream this graph's A^T chunks into SBUF ----
                at_t = []
                for j in range(NCH):
                    t = apool.tile([P, N], f32r, tag=f"at{j}", name=f"at{j}")
                    nc.sync.dma_start(
                        out=t[:], in_=at[(g * NCH + j) * P:(g * NCH + j + 1) * P, :])
                    at_t.append(t)

                # ---- gather features, transpose to feature-major xT ----
                xT = xpool.tile([P, N], f32r, tag="xT", name="xT")
                for c in range(NCH):
                    fnm = fpool.tile([P, D], f32, tag="fnm", name="fnm")
                    col = g * NCH + c
                    nc.gpsimd.indirect_dma_start(
                        out=fnm[:], out_offset=None, in_=table[:],
                        in_offset=bass.IndirectOffsetOnAxis(
                            ap=fidx_sb[:, col:col + 1], axis=0))
                    pst = tps.tile([P, P], f32, tag="tps", name="pst")
                    nc.tensor.transpose(out=pst[:], in_=fnm[:], identity=ident[:])
                    nc.vector.tensor_copy(out=xT[:, c * P:(c + 1) * P], in_=pst[:])

                # ---- residual branch: x1T = relu(wres.T @ xT + bres) ----
                x1T = xpool.tile([P, N], f32r, tag="x1T", name="x1T")
                for q in range(4):
                    ps_q = aps.tile([P, 512], f32, tag=f"aps{q}", name=f"psq{q}")
                    nc.tensor.matmul(out=ps_q[:], lhsT=wres_sb[:],
                                     rhs=xT[:, q * 512:(q + 1) * 512],
                                     start=True, stop=True)
                    nc.scalar.activation(
                        out=x1T[:, q * 512:(q + 1) * 512], in_=ps_q[:],
                        func=mybir.ActivationFunctionType.Relu, bias=bres_sb[:])

                # ---- GCN layers ----
                x_cur = xT
                for l in range(n_layers):
                    h = hpool.tile([P, N], f32r, tag="h", name="h")
                    for j in range(NCH):
                        ph = wps.tile([P, P], f32, tag="wps", name="ph")
                        nc.tensor.matmul(out=ph[:],
                                         lhsT=x_cur[:, j * P:(j + 1) * P],
                                         rhs=gw_sb[:, l * H:(l + 1) * H],
                                         start=True, stop=True)
                        nc.vector.tensor_copy(out=h[:, j * P:(j + 1) * P], in_=ph[:])
                    ps_l = [aps.tile([P, 512], f32, tag=f"aps{q}", name=f"apsl{q}") for q in range(4)]
                    for j in range(NCH):
                        hj = h[:, j * P:(j + 1) * P]
                        for q in range(4):
                            nc.tensor.matmul(
                                out=ps_l[q][:], lhsT=hj,
                                rhs=at_t[j][:, q * 512:(q + 1) * 512],
                                start=(j == 0), stop=(j == NCH - 1))
                    xn = xpool.tile([P, N], f32r, tag="xT", name="xn")
                    for q in range(4):
                        nc.scalar.activation(
                            out=xn[:, q * 512:(q + 1) * 512], in_=ps_l[q][:],
                            func=mybir.ActivationFunctionType.Relu,
                            bias=gb_sb[:, l:l + 1])
                    x_cur = xn

                # ---- fc1 with residual folded into PSUM accumulation ----
                fcq = fpool.tile([P, N], f32, tag="fcq", name="fcq", bufs=1)
                for q in range(4):
                    ps_q = aps.tile([P, 512], f32, tag=f"aps{q}", name=f"psq{q}")
                    nc.tensor.matmul(out=ps_q[:], lhsT=wfc_sb[:],
                                     rhs=x_cur[:, q * 512:(q + 1) * 512],
                                     start=True, stop=False)
                    nc.tensor.matmul(out=ps_q[:], lhsT=wfc_sb[:],
                                     rhs=x1T[:, q * 512:(q + 1) * 512],
                                     start=False, stop=True)
                    nc.scalar.activation(
                        out=fcq[:, q * 512:(q + 1) * 512], in_=ps_q[:],
                        func=mybir.ActivationFunctionType.Relu, bias=bfc_sb[:],
                        accum_out=macc[:, g * 4 + q:g * 4 + q + 1])

            # ---- head: means -> logits -> log_softmax ----
            nc.vector.tensor_reduce(
                out=means[:], in_=macc[:].rearrange("p (g q) -> p g q", q=4),
                axis=mybir.AxisListType.X, op=mybir.AluOpType.add)
            pl = wps.tile([GPC, O], f32, tag="wps", name="pl")
            nc.tensor.matmul(out=pl[:], lhsT=means[:], rhs=wlin_sb[:],
                             start=True, stop=True)
            lg_sb = const.tile([GPC, O], f32)
            nc.scalar.activation(out=lg_sb[:], in_=pl[:],
                                 func=mybir.ActivationFunctionType.Copy,
                                 scale=1.0 / N)
            nc.vector.tensor_add(out=lg_sb[:], in0=lg_sb[:], in1=lbb_sb[:])
            mx = const.tile([GPC, 1], f32)
            nc.vector.tensor_reduce(out=mx[:], in_=lg_sb[:],
                                    axis=mybir.AxisListType.X,
                                    op=mybir.AluOpType.max)
            tt = const.tile([GPC, O], f32)
            nc.vector.tensor_scalar(out=tt[:], in0=lg_sb[:], scalar1=mx[:],
                                    scalar2=None, op0=mybir.AluOpType.subtract)
            ex = const.tile([GPC, O], f32)
            nc.scalar.activation(out=ex[:], in_=tt[:],
                                 func=mybir.ActivationFunctionType.Exp)
            se = const.tile([GPC, 1], f32)
            nc.vector.tensor_reduce(out=se[:], in_=ex[:],
                                    axis=mybir.AxisListType.X,
                                    op=mybir.AluOpType.add)
            lse = const.tile([GPC, 1], f32)
            nc.scalar.activation(out=lse[:], in_=se[:],
                                 func=mybir.ActivationFunctionType.Ln)
            ls_sb = const.tile([GPC, O], f32)
            nc.vector.tensor_scalar(out=ls_sb[:], in0=tt[:], scalar1=lse[:],
                                    scalar2=None, op0=mybir.AluOpType.subtract)
            nc.sync.dma_start(out=out_lg[:], in_=lg_sb[:])
            nc.sync.dma_start(out=out_ls[:], in_=ls_sb[:])

    nc.compile()
    return nc


class _Runner:
    """Compile once, keep the jitted sharded executable for repeat calls."""

    def __init__(self, n_layers: int):
        import jax
        from jax.sharding import Mesh, PartitionSpec
        from jax.experimental.shard_map import shard_map

        self.jax = jax
        nc = _build_program(n_layers)
        self.nc = nc
        bass2jax.install_neuronx_cc_hook()

        in_names, out_names, out_avals, zero_outs = [], [], [], []
        pid_name = nc.partition_id_tensor.name if nc.partition_id_tensor else None
        for alloc in nc.m.functions[0].allocations:
            if not isinstance(alloc, mybir.MemoryLocationSet):
                continue
            name = alloc.memorylocations[0].name
            if alloc.kind == "ExternalInput":
                if name != pid_name:
                    in_names.append(name)
            elif alloc.kind == "ExternalOutput":
                out_names.append(name)
                shape = tuple(alloc.tensor_shape)
                dtype = mybir.dt.np(alloc.dtype)
                out_avals.append(jax.core.ShapedArray(shape, dtype))
                zero_outs.append(np.zeros(shape, dtype))
        self.in_names = list(in_names)
        self.out_names = out_names
        self.zero_outs = zero_outs
        n_params = len(in_names)
        all_names = in_names + out_names + ([pid_name] if pid_name else [])

        def _body(*args):
            operands = list(args)
            if pid_name is not None:
                operands.append(bass2jax.partition_id_tensor())
            return tuple(bass2jax._bass_exec_p.bind(
                *operands,
                out_avals=tuple(out_avals),
                in_names=tuple(all_names),
                out_names=tuple(out_names),
                lowering_input_output_aliases=(),
                sim_require_finite=True,
                sim_require_nnan=True,
                nc=nc,
            ))

        devices = jax.devices()[:N_CORES]
        mesh = Mesh(np.asarray(devices), ("core",))
        self.fn = jax.jit(
            shard_map(_body, mesh=mesh,
                      in_specs=(PartitionSpec("core"),) * (n_params + len(out_names)),
                      out_specs=(PartitionSpec("core"),) * len(out_names),
                      check_rep=False),
            keep_unused=True)

    def run(self, concat_inputs: list[np.ndarray]):
        jax = self.jax
        concat_zeros = [np.zeros((N_CORES * z.shape[0], *z.shape[1:]), z.dtype)
                        for z in self.zero_outs]
        outs = self.fn(*concat_inputs, *concat_zeros)
        jax.block_until_ready(outs)
        return {name: np.asarray(outs[i]) for i, name in enumerate(self.out_names)}


_RUNNERS: dict[int, _Runner] = {}


def _prepare_inputs(all_features, feature_index, edge_index,
                    lin_res_w, lin_res_b, gcn_w, gcn_b,
                    fc1_w, fc1_b, lin_w, lin_b, n_layers):
    """Build the concatenated (over cores, axis 0) device input list."""
    table = np.ascontiguousarray(all_features, np.float32)
    fi = np.asarray(feature_index).astype(np.int32).reshape(G, NCH, P)
    ei = np.asarray(edge_index).astype(np.int32)

    # fidx per core: [P, GPC*NCH]; fidx[p, g*NCH+c] = feature_index[g0+g, c*P+p]
    fidx_all = np.ascontiguousarray(
        fi.transpose(2, 0, 1)).reshape(P, G, NCH)        # [p, graph, chunk]

    # A^T per graph: accumulate duplicate (src,dst) cells, round each cell
    # value to the fp32r set, scatter into the dense matrix.  int32 keys +
    # np.put: int64 fancy indexing is pathologically slow in this numpy.
    at_all = np.zeros((G, N * N), np.float32)
    diag_keys = (np.arange(N, dtype=np.int64) * (N + 1)).astype(np.int32)
    for g in range(G):
        src = ei[g, 0]
        dst = ei[g, 1]
        deg = np.bincount(dst, minlength=N).astype(np.float32) + 1.0
        dinv = 1.0 / np.sqrt(deg)
        coef = dinv[src] * dinv[dst]
        keys = np.concatenate([src.astype(np.int32) * N + dst, diag_keys])
        vals = np.concatenate([coef, dinv * dinv]).astype(np.float64)
        order = np.argsort(keys, kind="stable")
        ks, vs = keys[order], vals[order]
        first = np.empty(len(ks), bool)
        first[0] = True
        first[1:] = ks[1:] != ks[:-1]
        starts = np.nonzero(first)[0]
        sums = np.add.reduceat(vs, starts).astype(np.float32)
        np.put(at_all[g], ks[starts], _round_f32r(sums))
    at_all = at_all.reshape(G, N, N)

    wres = _round_f32r(lin_res_w)
    gwr = _round_f32r(np.asarray(gcn_w, np.float32)[:n_layers])
    wfcr = _round_f32r(fc1_w)
    gbt = np.ascontiguousarray(np.asarray(gcn_b, np.float32)[:n_layers].T)
    bres = np.ascontiguousarray(np.asarray(lin_res_b, np.float32).reshape(H, 1))
    bfc = np.ascontiguousarray(np.asarray(fc1_b, np.float32).reshape(H, 1))
    wlin = np.ascontiguousarray(lin_w, np.float32)
    lbb = np.tile(np.asarray(lin_b, np.float32).reshape(1, O), (GPC, 1))

    per_core = {}
    per_core["table"] = [table] * N_CORES
    per_core["fidx"] = [np.ascontiguousarray(
        fidx_all[:, c * GPC:(c + 1) * GPC, :]).reshape(P, GPC * NCH)
        for c in range(N_CORES)]
    per_core["at"] = [at_all[c * GPC:(c + 1) * GPC].reshape(GPC * N, N)
                      for c in range(N_CORES)]
    for name, arr in [("wres", wres), ("bres", bres), ("gw", gwr),
                      ("gb", gbt), ("wfc", wfcr), ("bfc", bfc),
                      ("wlin", wlin), ("lbb", lbb)]:
        per_core[name] = [arr] * N_CORES
    return per_core


def kernel(all_features, feature_index, edge_index, action,
           lin_res_w, lin_res_b, gcn_w, gcn_b,
           fc1_w, fc1_b, lin_w, lin_b):
    n_layers = int(action) + 1
    assert 1 <= n_layers <= 3

    if n_layers not in _RUNNERS:
        _RUNNERS[n_layers] = _Runner(n_layers)
    runner = _RUNNERS[n_layers]

    per_core = _prepare_inputs(
        all_features, feature_index, edge_index,
        lin_res_w, lin_res_b, gcn_w, gcn_b, fc1_w, fc1_b, lin_w, lin_b,
        n_layers)

    concat = [np.concatenate(per_core[name], axis=0)
              for name in runner.in_names]
    outs = runner.run(concat)
    ls = outs["out_ls"].reshape(N_CORES, GPC, O).reshape(G, O)
    lg = outs["out_lg"].reshape(N_CORES, GPC, O).reshape(G, O)
    return np.asarray(ls, np.float32), np.asarray(lg, np.float32)



# revision 3
# speedup vs baseline: 3.8692x; 3.8692x over previous
"""GCN (3-layer, PyG GCNConv-style) forward on 8 Trainium2 NeuronCores.

Strategy: data-parallel over the 64 graphs (8 graphs per core).  The
message-passing scatter-add is a dense normalized-adjacency matmul run in
fp8e4m3 with MatmulPerfMode.DoubleRow (two 128-row k-tiles per instruction,
0.5 cycles/row), which is 4x the fp32r FLOP rate for the dominant A@h
product.  Weight-side matmuls stay bf16 (weight quantization error is
systematic across nodes and does not average out; fp8 weights blow the
error budget, bf16 is ~1.6e-3 on the logits).

Host-side prep: the feature gather from the 500k-row table and the dense
A^T build happen on the host; the device receives per-graph feature tiles
(bf16, feature-major [128, 2048]) and A^T tiles (fp8, [128 src-part,
16 src-chunk, 2048 dst] swizzle) so each graph needs exactly two large
contiguous DMAs.  Per layer on device:
    h   = x @ W        (16 bf16 matmuls, 4-chunk PSUM groups, bulk-cast
                        to fp8 on DVE/Pool)
    x'  = relu(A @ h + b)  (32 fp8 DoubleRow matmuls into 4 psum strips,
                        relu+bias on ACT writing bf16)
The layer orientations alternate (feat-major <-> node-major) so no
transposes are needed anywhere.
"""

import os
import sys

for _p in ("/opt/trn_rl_repo", "/root/.axon_site/_ro/trn_rl_repo"):
    if os.path.isdir(_p) and _p not in sys.path:
        sys.path.insert(0, _p)

import numpy as np
import ml_dtypes

import concourse.bass as bass
import concourse.bacc as bacc
import concourse.mybir as mybir
import concourse.tile as tile
from concourse import bass2jax

G, N, E = 64, 2048, 32768
D = H = 128
O = 2
ALL = 500_000
P = 128
N_CORES = 8
GPC = G // N_CORES          # graphs per core
NCH = N // P                # 128-row chunks per graph (16)

f32 = mybir.dt.float32
bf16 = mybir.dt.bfloat16
f8 = mybir.dt.float8e4
i32 = mybir.dt.int32

E4NP = ml_dtypes.float8_e4m3      # == mybir.dt.np(float8e4)
BFNP = ml_dtypes.bfloat16

DR = mybir.MatmulPerfMode.DoubleRow
RELU = mybir.ActivationFunctionType.Relu


def _build_program(n_layers: int):
    nc = bacc.Bacc("TRN2", target_bir_lowering=False, debug=False,
                   num_devices=N_CORES)

    x0 = nc.dram_tensor("x0", [GPC * P, N], bf16, kind="ExternalInput")
    at = nc.dram_tensor("at", [GPC * P, NCH * N], f8, kind="ExternalInput")
    wres = nc.dram_tensor("wres", [D, H], bf16, kind="ExternalInput")
    bres = nc.dram_tensor("bres", [H, 1], f32, kind="ExternalInput")
    gw = nc.dram_tensor("gw", [n_layers, H, H], bf16, kind="ExternalInput")
    gb = nc.dram_tensor("gb", [H, n_layers], f32, kind="ExternalInput")
    wfc = nc.dram_tensor("wfc", [H, H], bf16, kind="ExternalInput")
    bfc = nc.dram_tensor("bfc", [H, 1], f32, kind="ExternalInput")
    wlin = nc.dram_tensor("wlin", [H, O], f32, kind="ExternalInput")
    lbb = nc.dram_tensor("lbb", [GPC, O], f32, kind="ExternalInput")
    out_ls = nc.dram_tensor("out_ls", [GPC, O], f32, kind="ExternalOutput")
    out_lg = nc.dram_tensor("out_lg", [GPC, O], f32, kind="ExternalOutput")

    with tile.TileContext(nc) as tc:
        with tc.tile_pool(name="const", bufs=1) as const, \
             tc.tile_pool(name="apool", bufs=2) as apool, \
             tc.tile_pool(name="xpool", bufs=2) as xpool, \
             tc.tile_pool(name="x1pool", bufs=2) as x1pool, \
             tc.tile_pool(name="hpool", bufs=2) as hpool, \
             tc.tile_pool(name="fpool", bufs=2) as fpool, \
             tc.tile_pool(name="hps", bufs=2, space="PSUM") as hps, \
             tc.tile_pool(name="wps", bufs=1, space="PSUM") as wps, \
             tc.tile_pool(name="aps", bufs=1, space="PSUM") as aps:

            # ---- constants ----
            wres_sb = const.tile([D, H], bf16)
            nc.sync.dma_start(out=wres_sb[:], in_=wres[:])
            gw_sb = const.tile([H, n_layers * H], bf16)
            for l in range(n_layers):
                nc.sync.dma_start(out=gw_sb[:, l * H:(l + 1) * H], in_=gw[l])
            gb_sb = const.tile([H, n_layers], f32)
            nc.sync.dma_start(out=gb_sb[:], in_=gb[:])
            bres_sb = const.tile([H, 1], f32)
            nc.sync.dma_start(out=bres_sb[:], in_=bres[:])
            wfc_sb = const.tile([H, H], bf16)
            nc.sync.dma_start(out=wfc_sb[:], in_=wfc[:])
            bfc_sb = const.tile([H, 1], f32)
            nc.sync.dma_start(out=bfc_sb[:], in_=bfc[:])
            wlin_sb = const.tile([H, O], f32)
            nc.sync.dma_start(out=wlin_sb[:], in_=wlin[:])
            lbb_sb = const.tile([GPC, O], f32)
            nc.sync.dma_start(out=lbb_sb[:], in_=lbb[:])
            macc = const.tile([P, GPC * 4], f32)
            means = const.tile([P, GPC], f32)

            for g in range(GPC):
                # ---- two big DMAs: A^T (fp8 swizzle) and features ----
                at_sb = apool.tile([P, NCH, N], f8, tag="at", name=f"at{g}")
                nc.sync.dma_start(
                    out=at_sb[:],
                    in_=at[g * P:(g + 1) * P, :].rearrange(
                        "p (s n) -> p s n", s=NCH))
                xT = xpool.tile([P, N], bf16, tag="x", name=f"x0_{g}")
                nc.sync.dma_start(out=xT[:], in_=x0[g * P:(g + 1) * P, :])

                # ---- residual branch: x1T = relu(wres.T @ xT + bres) ----
                x1T = x1pool.tile([P, N], bf16, tag="x1", name=f"x1_{g}")
                for q in range(4):
                    ps_q = aps.tile([P, 512], f32, tag=f"aps{q}", name=f"rs{q}")
                    nc.tensor.matmul(out=ps_q[:], lhsT=wres_sb[:],
                                     rhs=xT[:, q * 512:(q + 1) * 512],
                                     start=True, stop=True)
                    nc.scalar.activation(
                        out=x1T[:, q * 512:(q + 1) * 512], in_=ps_q[:],
                        func=RELU, bias=bres_sb[:])

                # ---- GCN layers (feat-major in, feat-major out) ----
                x_cur = xT
                for l in range(n_layers):
                    h8 = hpool.tile([P, NCH, H], f8, tag="h", name=f"h{g}_{l}")
                    # h = x @ W in 4-chunk psum groups, bulk-cast to fp8
                    for jj in range(4):
                        hp = hps.tile([P, 512], f32, tag="hps",
                                      name=f"hp{g}_{l}_{jj}")
                        for c in range(4):
                            j = jj * 4 + c
                            nc.tensor.matmul(
                                out=hp[:, c * H:(c + 1) * H],
                                lhsT=x_cur[:, j * P:(j + 1) * P],
                                rhs=gw_sb[:, l * H:(l + 1) * H],
                                start=(c == 0), stop=(c == 3))
                        nc.vector.tensor_copy(
                            out=h8[:, jj * 4:(jj + 1) * 4, :].rearrange(
                                "p s f -> p (s f)"),
                            in_=hp[:])
                    # x' = relu(A @ h + b): fp8 DoubleRow, k-pair outer so
                    # strips start as soon as the first h-group lands
                    ps_l = [aps.tile([P, 512], f32, tag=f"aps{q}",
                                     name=f"as{g}_{l}_{q}") for q in range(4)]
                    for j in range(NCH // 2):
                        hj = h8[:, 2 * j:2 * j + 2, :]
                        for q in range(4):
                            nc.tensor.matmul(
                                out=ps_l[q][:], lhsT=hj,
                                rhs=at_sb[:, 2 * j:2 * j + 2,
                                          q * 512:(q + 1) * 512],
                                start=(j == 0), stop=(j == NCH // 2 - 1),
                                perf_mode=DR)
                    xn = xpool.tile([P, N], bf16, tag="x", name=f"x{g}_{l}")
                    for q in range(4):
                        nc.scalar.activation(
                            out=xn[:, q * 512:(q + 1) * 512], in_=ps_l[q][:],
                            func=RELU, bias=gb_sb[:, l:l + 1])
                    x_cur = xn

                # ---- fc1 with residual folded into PSUM accumulation ----
                fcq = fpool.tile([P, 512], f32, tag="fcq", name=f"fc{g}")
                for q in range(4):
                    ps_q = aps.tile([P, 512], f32, tag=f"aps{q}", name=f"fs{q}")
                    nc.tensor.matmul(out=ps_q[:], lhsT=wfc_sb[:],
                                     rhs=x_cur[:, q * 512:(q + 1) * 512],
                                     start=True, stop=False)
                    nc.tensor.matmul(out=ps_q[:], lhsT=wfc_sb[:],
                                     rhs=x1T[:, q * 512:(q + 1) * 512],
                                     start=False, stop=True)
                    nc.scalar.activation(
                        out=fcq[:], in_=ps_q[:],
                        func=RELU, bias=bfc_sb[:],
                        accum_out=macc[:, g * 4 + q:g * 4 + q + 1])

            # ---- head: means -> logits -> log_softmax ----
            nc.vector.tensor_reduce(
                out=means[:], in_=macc[:].rearrange("p (g q) -> p g q", q=4),
                axis=mybir.AxisListType.X, op=mybir.AluOpType.add)
            pl = wps.tile([GPC, O], f32, tag="wps", name="pl")
            nc.tensor.matmul(out=pl[:], lhsT=means[:], rhs=wlin_sb[:],
                             start=True, stop=True)
            lg_sb = const.tile([GPC, O], f32)
            nc.scalar.activation(out=lg_sb[:], in_=pl[:],
                                 func=mybir.ActivationFunctionType.Copy,
                                 scale=1.0 / N)
            nc.vector.tensor_add(out=lg_sb[:], in0=lg_sb[:], in1=lbb_sb[:])
            mx = const.tile([GPC, 1], f32)
            nc.vector.tensor_reduce(out=mx[:], in_=lg_sb[:],
                                    axis=mybir.AxisListType.X,
                                    op=mybir.AluOpType.max)
            tt = const.tile([GPC, O], f32)
            nc.vector.tensor_scalar(out=tt[:], in0=lg_sb[:], scalar1=mx[:],
                                    scalar2=None, op0=mybir.AluOpType.subtract)
            ex = const.tile([GPC, O], f32)
            nc.scalar.activation(out=ex[:], in_=tt[:],
                                 func=mybir.ActivationFunctionType.Exp)
            se = const.tile([GPC, 1], f32)
            nc.vector.tensor_reduce(out=se[:], in_=ex[:],
                                    axis=mybir.AxisListType.X,
                                    op=mybir.AluOpType.add)
            lse = const.tile([GPC, 1], f32)
            nc.scalar.activation(out=lse[:], in_=se[:],
                                 func=mybir.ActivationFunctionType.Ln)
            ls_sb = const.tile([GPC, O], f32)
            nc.vector.tensor_scalar(out=ls_sb[:], in0=tt[:], scalar1=lse[:],
                                    scalar2=None, op0=mybir.AluOpType.subtract)
            nc.sync.dma_start(out=out_lg[:], in_=lg_sb[:])
            nc.sync.dma_start(out=out_ls[:], in_=ls_sb[:])

    nc.compile()
    return nc


class _Runner:
    """Compile once, keep the jitted sharded executable for repeat calls."""

    def __init__(self, n_layers: int):
        import jax
        from jax.sharding import Mesh, PartitionSpec
        from jax.experimental.shard_map import shard_map

        self.jax = jax
        nc = _build_program(n_layers)
        self.nc = nc
        bass2jax.install_neuronx_cc_hook()

        in_names, out_names, out_avals, zero_outs = [], [], [], []
        pid_name = nc.partition_id_tensor.name if nc.partition_id_tensor else None
        for alloc in nc.m.functions[0].allocations:
            if not isinstance(alloc, mybir.MemoryLocationSet):
                continue
            name = alloc.memorylocations[0].name
            if alloc.kind == "ExternalInput":
                if name != pid_name:
                    in_names.append(name)
            elif alloc.kind == "ExternalOutput":
                out_names.append(name)
                shape = tuple(alloc.tensor_shape)
                dtype = mybir.dt.np(alloc.dtype)
                out_avals.append(jax.core.ShapedArray(shape, dtype))
                zero_outs.append(np.zeros(shape, dtype))
        self.in_names = list(in_names)
        self.out_names = out_names
        self.zero_outs = zero_outs
        n_params = len(in_names)
        all_names = in_names + out_names + ([pid_name] if pid_name else [])

        def _body(*args):
            operands = list(args)
            if pid_name is not None:
                operands.append(bass2jax.partition_id_tensor())
            return tuple(bass2jax._bass_exec_p.bind(
                *operands,
                out_avals=tuple(out_avals),
                in_names=tuple(all_names),
                out_names=tuple(out_names),
                lowering_input_output_aliases=(),
                sim_require_finite=True,
                sim_require_nnan=True,
                nc=nc,
            ))

        devices = jax.devices()[:N_CORES]
        mesh = Mesh(np.asarray(devices), ("core",))
        self.fn = jax.jit(
            shard_map(_body, mesh=mesh,
                      in_specs=(PartitionSpec("core"),) * (n_params + len(out_names)),
                      out_specs=(PartitionSpec("core"),) * len(out_names),
                      check_rep=False),
            keep_unused=True)

    def run(self, concat_inputs: list[np.ndarray]):
        jax = self.jax
        concat_zeros = [np.zeros((N_CORES * z.shape[0], *z.shape[1:]), z.dtype)
                        for z in self.zero_outs]
        outs = self.fn(*concat_inputs, *concat_zeros)
        jax.block_until_ready(outs)
        return {name: np.asarray(outs[i]) for i, name in enumerate(self.out_names)}


_RUNNERS: dict[int, _Runner] = {}


def _prepare_inputs(all_features, feature_index, edge_index,
                    lin_res_w, lin_res_b, gcn_w, gcn_b,
                    fc1_w, fc1_b, lin_w, lin_b, n_layers):
    """Build the concatenated (over cores, axis 0) device input list."""
    feats = np.ascontiguousarray(all_features, np.float32)
    fi = np.asarray(feature_index).astype(np.int64)
    ei = np.asarray(edge_index).astype(np.int32)

    # host gather + transpose to feature-major bf16 [G, 128, 2048]
    x0_all = np.ascontiguousarray(
        feats[fi].transpose(0, 2, 1)).astype(BFNP)          # [G, D, N]

    # A^T per graph: accumulate duplicate (src,dst) cells, quantize fp8,
    # swizzle to [128 part, 16 chunk, 2048 dst].
    at_all = np.zeros((G, N * N), np.float32)
    diag_keys = (np.arange(N, dtype=np.int64) * (N + 1)).astype(np.int32)
    for g in range(G):
        src = ei[g, 0]
        dst = ei[g, 1]
        deg = np.bincount(dst, minlength=N).astype(np.float32) + 1.0
        dinv = 1.0 / np.sqrt(deg)
        coef = dinv[src] * dinv[dst]
        keys = np.concatenate([src.astype(np.int32) * N + dst, diag_keys])
        vals = np.concatenate([coef, dinv * dinv]).astype(np.float64)
        order = np.argsort(keys, kind="stable")
        ks, vs = keys[order], vals[order]
        first = np.empty(len(ks), bool)
        first[0] = True
        first[1:] = ks[1:] != ks[:-1]
        starts = np.nonzero(first)[0]
        sums = np.add.reduceat(vs, starts).astype(np.float32)
        np.put(at_all[g], ks[starts], sums)
    at8 = at_all.reshape(G, NCH, P, N).transpose(0, 2, 1, 3)  # [G,128,16,2048]
    at8 = np.ascontiguousarray(at8).astype(E4NP).reshape(G, P, NCH * N)

    wres = np.asarray(lin_res_w, np.float32).astype(BFNP)
    gwr = np.asarray(gcn_w, np.float32)[:n_layers].astype(BFNP)
    wfcr = np.asarray(fc1_w, np.float32).astype(BFNP)
    gbt = np.ascontiguousarray(np.asarray(gcn_b, np.float32)[:n_layers].T)
    bres = np.ascontiguousarray(np.asarray(lin_res_b, np.float32).reshape(H, 1))
    bfc = np.ascontiguousarray(np.asarray(fc1_b, np.float32).reshape(H, 1))
    wlin = np.ascontiguousarray(lin_w, np.float32)
    lbb = np.tile(np.asarray(lin_b, np.float32).reshape(1, O), (GPC, 1))

    per_core = {}
    per_core["x0"] = [x0_all[c * GPC:(c + 1) * GPC].reshape(GPC * P, N)
                      for c in range(N_CORES)]
    per_core["at"] = [at8[c * GPC:(c + 1) * GPC].reshape(GPC * P, NCH * N)
                      for c in range(N_CORES)]
    for name, arr in [("wres", wres), ("bres", bres), ("gw", gwr),
                      ("gb", gbt), ("wfc", wfcr), ("bfc", bfc),
                      ("wlin", wlin), ("lbb", lbb)]:
        per_core[name] = [arr] * N_CORES
    return per_core


def kernel(all_features, feature_index, edge_index, action,
           lin_res_w, lin_res_b, gcn_w, gcn_b,
           fc1_w, fc1_b, lin_w, lin_b):
    n_layers = int(action) + 1
    assert 1 <= n_layers <= 3

    if n_layers not in _RUNNERS:
        _RUNNERS[n_layers] = _Runner(n_layers)
    runner = _RUNNERS[n_layers]

    per_core = _prepare_inputs(
        all_features, feature_index, edge_index,
        lin_res_w, lin_res_b, gcn_w, gcn_b, fc1_w, fc1_b, lin_w, lin_b,
        n_layers)

    concat = [np.concatenate(per_core[name], axis=0)
              for name in runner.in_names]
    outs = runner.run(concat)
    ls = outs["out_ls"].reshape(N_CORES, GPC, O).reshape(G, O)
    lg = outs["out_lg"].reshape(N_CORES, GPC, O).reshape(G, O)
    return np.asarray(ls, np.float32), np.asarray(lg, np.float32)
